# revision 41
# baseline (speedup 1.0000x reference)
"""DiceLoss (multiclass, softmax over C=16) on 8 Trainium2 NeuronCores.

Data-parallel: batch b -> core b. Per core, logits [16, 512*512] are packed
as [128, 32768] fp8-e4m3 (softmax is insensitive to small logit noise; the
quantization error averages out over 16k+ pixels per class): partition
p = g*16 + c (g = pixel-group of 32768 pixels, c = class), free axis =
pixel-within-group. Per 2048-pixel chunk:

  E  = exp(L)                 ACT (the ONLY ScalarE op -> one table set)
  D  = SelRep.T @ E           PE -> PSUM f32 (SelRep = 16x16 block-diag ones
                              -> per-pixel softmax denominator, replicated to
                              all 16 class-partitions; constant weights)
  DT[8i:8i+8] = W8.T @ E      PE -> persistent PSUM f32 [128, 2048]: compact
                              per-pixel denominators, pixel-major (row i*8+g
                              holds group g's pixels of chunk i). Same E
                              stream, W8 = group-selector ones [128, 8].
  P  = E * approx(1/D)        custom DVE op RECIP_MUL_DICE, one instruction:
       p_sum += sum(P)        bitcast-NOT exponent-flip seed + minimax-linear
                              refine (~1.8e-3 rel err, cancels in the dice
                              ratio), fused in1 multiply + free-axis accum
                              (reads D straight from PSUM). Output tensor is
                              scratch -- only the accumulator is used.

Intersection needs only the TARGET-class probability per pixel. The host
sends lt [128, 2048] fp8 = the target-class logit per pixel (same fp8
values as in xp, pixel-major, row i*8+g). Tail:

  ET  = exp(lt)               ACT (bit-identical to the matching E entries)
  ITR = ET * approx(1/DT)     one more RECIP_MUL_DICE [128, 2048] -> per-pixel
                              target-class softmax prob, DMA'd back (512KB).

Host folds: p_sum from the accumulator columns, I_c = bincount(targets,
weights=ITR), t_sum = bincount(targets). No mask tensor, no mask multiply,
no intersection matmul stream. No on-device collective: per-core partials
combine on host.
"""

import sys

for _p in ("/opt/trn_rl_repo",):
    if _p not in sys.path:
        sys.path.insert(0, _p)

from operator import add

import numpy as np
import ml_dtypes

import concourse.bacc as bacc
import concourse.bass as bass
import concourse.dve_ops as dve_ops
import concourse.tile as tile
from concourse import mybir
from concourse.bass_utils import run_bass_kernel_spmd
from concourse.dve_ops import DveOp
from concourse.dve_spec import (
    AluOp,
    Bin,
    C0,
    C1,
    Spec,
    Src0,
    Src1,
    Zero,
    _has_src1,
    lower,
)
from concourse.dve_uop import DveOpSpec

BF16 = ml_dtypes.bfloat16
F8 = ml_dtypes.float8_e4m3fn

B, C, H, W = 8, 16, 512, 512
HW = H * W           # 262144 pixels per batch/core
G = 8                # pixel groups per core
M = HW // G          # 32768 pixels per group (free-dim length)
P = G * C            # 128 partitions
NCHUNK = 16
N = M // NCHUNK      # 2048 pixels per outer tile (DMA/exp granularity)
NH = 1024            # pixels per PSUM-bound inner chunk
NHPER = N // NH      # inner chunks per outer tile
SMOOTH = 1.0
IGNORE_INDEX = 255

# minimax-linear fit of 1/t on [-4.5, -4] (the interval x*bitcast(~x) lands
# in for any positive fp32 x); relative error 1.81e-3
RECIP_A = -0.47108412121536725
RECIP_B = -0.05538388804827088

_CACHE: dict = {}


def _ref_recip_mul(in0, in1, c0, c1, c2):
    u = (~np.asarray(in0, np.float32).view(np.int32)).view(np.float32)
    t = (in0 * u).astype(np.float32)
    b = ((u * (c0 + c1 * t)) * in1).astype(np.float32)
    return b, b.reshape(b.shape[0], -1).sum(axis=-1, keepdims=True)


def _make_dve_op(name, spec):
    """Build a DveOp with computed uop shas and register it in dve_ops."""
    if name in dve_ops._SUB_OPCODE_FOR_NAME:
        return next(op for op in dve_ops.OPS if op.name == name)
    shas = {}
    for ver in ("v3", "v4"):
        tmp = DveOpSpec(
            name=name, opcode=0, uops=lower(spec, ver=ver), rd1_en=_has_src1(spec)
        )
        shas[ver] = tmp.sha(ver)
    op = DveOp(name, spec, subdim=False, uops_sha=shas)
    row = dve_ops._CUSTOM_DVE_ROW_BASE + len(dve_ops.OPS)
    assert row < 0x20
    dve_ops.OPS.append(op)
    dve_ops._SUB_OPCODE_FOR_NAME[name] = row
    dve_ops.CUSTOM_DVE_SPECS[name] = spec
    return op


_u = Bin(AluOp.BITWISE_NOT, Src0, Src0)
_t = Src0 * _u

RECIP_MUL_DICE = _make_dve_op(
    "RECIP_MUL_DICE",
    Spec(
        body=(_u * (C0 + C1 * _t)) * Src1,
        accum=add,
        accum_init=Zero,
        reference=_ref_recip_mul,
    ),
)


def _build():
    nc = bacc.Bacc("TRN2", target_bir_lowering=False, debug=False)
    bf = mybir.dt.bfloat16
    f32 = mybir.dt.float32

    f8 = mybir.dt.float8e4
    xp = nc.dram_tensor("xp", (P, M), f8, kind="ExternalInput").ap()
    lt = nc.dram_tensor("lt", (P, N), f8, kind="ExternalInput").ap()
    sel = nc.dram_tensor("sel", (P, P), bf, kind="ExternalInput").ap()
    # Group-selector ones [128, 8] embedded at columns 24..32 of a zero
    # [128, 56] band; the 32-wide window starting at col 24-8k is the
    # chunk-(4q+k) DT weight variant (selector at window cols 8k..8k+8, so
    # DT rows land at partition 32q+8k+g; PE out base must be 0/32/64/96
    # and the 4-chunk group accumulates into one 32-row PSUM block).
    w8 = nc.dram_tensor("w8", (P, 56), bf, kind="ExternalInput").ap()
    itr = nc.dram_tensor("itr", (P, N), bf, kind="ExternalOutput").ap()
    NACC = NHPER * NCHUNK
    accp = nc.dram_tensor("accp", (P, NACC), f32, kind="ExternalOutput").ap()

    with tile.TileContext(nc) as tc:
        with (
            tc.tile_pool(name="io", bufs=4) as io,
            tc.tile_pool(name="wt", bufs=1) as wt,
            tc.tile_pool(name="ps", bufs=2, space=bass.MemorySpace.PSUM) as ps,
            tc.tile_pool(name="pdt", bufs=1, space=bass.MemorySpace.PSUM) as pdt,
            tc.tile_pool(name="ac", bufs=1) as ac,
        ):
            selt = wt.tile([P, P], bf)
            w8t = wt.tile([P, 56], bf)
            ltt = wt.tile([P, N], f8)
            ET = ac.tile([P, N], bf)
            ITR = ac.tile([P, N], bf)
            scr = ac.tile([P, 1], f32)

            accP = ac.tile([P, NACC], f32)
            DTp = pdt.tile([P, N], f32)

            def dt_matmul(i, E, half=None):
                # compact per-pixel denominators for this chunk's pixels:
                # DTp[i*8+g, j] = sum_c E[g*16+c, j]. The 32-row block
                # 32*(i//4) accumulates chunks 4q..4q+3 via shifted zero-
                # padded weight columns (PE out base must be 0/32/64/96).
                q, k = divmod(i, 4)
                rng = (
                    range(0, N, 512)
                    if half is None
                    else range(half * NH, (half + 1) * NH, 512)
                )
                for s in rng:
                    nc.tensor.matmul(
                        DTp[32 * q : 32 * (q + 1), s : s + 512],
                        w8t[:, 24 - 8 * k : 56 - 8 * k],
                        E[:, s : s + 512],
                        start=(k == 0),
                        stop=(k == 3),
                        tile_position=(0, 32 * q),
                    )

            def recips(i, E, interleave_dt=True):
                # per EXP-half: 2 denominator matmuls + 2 DT matmuls back to
                # back on the PE (denser PE stream, hotter p-state), then the
                # fused recip+p_sum op on Vector
                for h in range(NHPER):
                    D = ps.tile([P, NH], f32, tag="D")
                    for s2 in range(0, NH, 512):
                        nc.tensor.matmul(
                            D[:, s2 : s2 + 512],
                            selt[:],
                            E[:, h * NH + s2 : h * NH + s2 + 512],
                            start=True,
                            stop=True,
                        )
                    if interleave_dt:
                        dt_matmul(i, E, half=h)
                    Pt = io.tile([P, NH], bf, tag="P")
                    nc.vector._custom_dve(
                        RECIP_MUL_DICE,
                        out=Pt[:],
                        in0=D[:],
                        in1=E[:, h * NH : (h + 1) * NH],
                        s0=RECIP_A,
                        s1=RECIP_B,
                        accum_out=accP[:, NHPER * i + h : NHPER * i + h + 1],
                    )

            for i in range(NCHUNK):
                sl = slice(i * N, (i + 1) * N)
                # halved DMA + EXP: shortens the produce->consume latency at
                # each pipeline stage (PE can start on h0 while h1 still exps)
                L = io.tile([P, N], f8, tag="L", bufs=8)
                if i == 0:
                    # chunk 0 leads the GpSimd ring (shortest preamble) so
                    # its halves hit the HW queues first; weights lead the
                    # Sync ring; Scalar ring stays clean for the table load
                    nc.gpsimd.dma_start(L[:, 0:NH], xp[:, 0:NH])
                    nc.gpsimd.dma_start(L[:, NH:N], xp[:, NH:N])
                    nc.sync.dma_start(selt[:], sel)
                    nc.sync.dma_start(w8t[:], w8)
                else:
                    nc.sync.dma_start(L[:, 0:NH], xp[:, i * N : i * N + NH])
                    nc.gpsimd.dma_start(
                        L[:, NH:N], xp[:, i * N + NH : (i + 1) * N]
                    )
                    if i == 1:
                        nc.gpsimd.dma_start(ltt[:], lt)

                E = io.tile([P, N], bf, tag="E", bufs=8)
                nc.scalar.activation(
                    E[:, 0:NH], L[:, 0:NH], mybir.ActivationFunctionType.Exp
                )
                nc.scalar.activation(
                    E[:, NH:N], L[:, NH:N], mybir.ActivationFunctionType.Exp
                )
                if i == 2:
                    # target-logit exp: ScalarE has slack mid-pipeline
                    nc.scalar.activation(
                        ET[:], ltt[:], mybir.ActivationFunctionType.Exp
                    )

                if i < NCHUNK - 1:
                    recips(i, E)
                    if i == NCHUNK - 2:
                        # bulk of the p_sum partials ships while chunk 15
                        # runs; only 2 columns remain on the critical tail
                        nc.gpsimd.dma_start(
                            accp[:, 0 : NHPER * (NCHUNK - 1)],
                            accP[:, 0 : NHPER * (NCHUNK - 1)],
                        )
                else:
                    # last chunk: DTp completes first, then ITR + its DMA
                    # hide behind the final pair of recip ops
                    dt_matmul(i, E)
                    nc.vector._custom_dve(
                        RECIP_MUL_DICE,
                        out=ITR[:],
                        in0=DTp[:],
                        in1=ET[:],
                        s0=RECIP_A,
                        s1=RECIP_B,
                        accum_out=scr[:],
                    )
                    nc.gpsimd.dma_start(itr, ITR[:])
                    recips(i, E, interleave_dt=False)

            nc.gpsimd.dma_start(
                accp[:, NHPER * (NCHUNK - 1) :], accP[:, NHPER * (NCHUNK - 1) :]
            )

    nc.compile()
    return nc


def _get_nc():
    nc = _CACHE.get("nc")
    if nc is None:
        nc = _build()
        _CACHE["nc"] = nc
    return nc


def _host_inputs(logits, targets):
    sel_np = np.kron(
        np.eye(G, dtype=np.float32), np.ones((C, C), np.float32)
    ).astype(BF16)  # [128, 128] block-diag 16x16 ones
    w8_np = np.zeros((P, 56), np.float32)
    w8_np[:, 24:32] = np.kron(
        np.eye(G, dtype=np.float32), np.ones((C, 1), np.float32)
    )  # selector band: w8[g*16+c, 24+g] = 1
    w8_np = w8_np.astype(BF16)

    logits = np.asarray(logits)
    targets = np.asarray(targets)
    in_maps = []
    for b in range(B):
        lf = logits[b].reshape(C, HW)
        xp = lf.reshape(C, G, M).transpose(1, 0, 2).reshape(P, M).astype(F8)
        tflat = targets[b].reshape(HW).astype(np.int64)
        tsafe = np.where(tflat == IGNORE_INDEX, 0, tflat)
        # target-class logit per pixel, pixel-major rows q = i*8 + g
        ltf = lf[tsafe, np.arange(HW)]
        ltp = (
            ltf.reshape(G, NCHUNK, N).transpose(1, 0, 2).reshape(P, N).astype(F8)
        )
        in_maps.append({"xp": xp, "lt": ltp, "sel": sel_np, "w8": w8_np})
    return in_maps


def _combine(results, targets):
    targets = np.asarray(targets)
    Ps = np.zeros(C, np.float64)
    Ic = np.zeros(C, np.float64)
    Ts = np.zeros(C, np.float64)
    for b, r in enumerate(results):
        acc = r["accp"].astype(np.float64)  # [128, 32] p_sum partials
        Ps += acc.sum(axis=1).reshape(G, C).sum(axis=0)
        # itr rows q = i*8 + g, cols j -> pixel g*M + i*N + j
        itr = (
            r["itr"].astype(np.float64).reshape(NCHUNK, G, N).transpose(1, 0, 2)
        ).reshape(HW)
        tflat = targets[b].reshape(HW).astype(np.int64)
        valid = tflat != IGNORE_INDEX
        Ic += np.bincount(tflat[valid], weights=itr[valid], minlength=C)[:C]
        Ts += np.bincount(tflat[valid], minlength=C)[:C]
    dice = (2.0 * Ic + SMOOTH) / (Ps + Ts + SMOOTH)
    return np.float32(np.mean(1.0 - dice))


def kernel(logits, targets):
    nc = _get_nc()
    in_maps = _host_inputs(logits, targets)
    res = run_bass_kernel_spmd(nc, in_maps, list(range(B)))
    return _combine(res.results, targets)


if __name__ == "__main__":
    rng = np.random.default_rng(0)
    logits = rng.standard_normal((B, C, H, W), dtype=np.float32)
    targets = rng.integers(0, C, size=(B, H, W)).astype(np.int64)
    print("loss:", kernel(logits, targets))


# revision 42
# speedup vs baseline: 1.0099x; 1.0099x over previous
"""DiceLoss (multiclass, softmax over C=16) on 8 Trainium2 NeuronCores.

Data-parallel: batch b -> core b. Per core, logits [16, 512*512] are packed
as [128, 32768] fp8-e4m3 (softmax is insensitive to small logit noise; the
quantization error averages out over 16k+ pixels per class): partition
p = g*16 + c (g = pixel-group of 32768 pixels, c = class), free axis =
pixel-within-group. Per 2048-pixel chunk:

  E  = exp(L)                 ACT (the ONLY ScalarE op -> one table set)
  D  = SelRep.T @ E           PE -> PSUM f32 (SelRep = 16x16 block-diag ones
                              -> per-pixel softmax denominator, replicated to
                              all 16 class-partitions; constant weights)
  DT[8i:8i+8] = W8.T @ E      PE -> persistent PSUM f32 [128, 2048]: compact
                              per-pixel denominators, pixel-major (row i*8+g
                              holds group g's pixels of chunk i). Same E
                              stream, W8 = group-selector ones [128, 8].
  P  = E * approx(1/D)        custom DVE op RECIP_MUL_DICE, one instruction:
       p_sum += sum(P)        bitcast-NOT exponent-flip seed + minimax-linear
                              refine (~1.8e-3 rel err, cancels in the dice
                              ratio), fused in1 multiply + free-axis accum
                              (reads D straight from PSUM). Output tensor is
                              scratch -- only the accumulator is used.

Intersection needs only the TARGET-class probability per pixel. The host
sends lt [128, 2048] fp8 = the target-class logit per pixel (same fp8
values as in xp, pixel-major, row i*8+g). Tail:

  ET  = exp(lt)               ACT (bit-identical to the matching E entries)
  ITR = ET * approx(1/DT)     one more RECIP_MUL_DICE [128, 2048] -> per-pixel
                              target-class softmax prob, DMA'd back (512KB).

Host folds: p_sum from the accumulator columns, I_c = bincount(targets,
weights=ITR), t_sum = bincount(targets). No mask tensor, no mask multiply,
no intersection matmul stream. No on-device collective: per-core partials
combine on host.
"""

import sys

for _p in ("/opt/trn_rl_repo",):
    if _p not in sys.path:
        sys.path.insert(0, _p)

from operator import add

import numpy as np
import ml_dtypes

import concourse.bacc as bacc
import concourse.bass as bass
import concourse.dve_ops as dve_ops
import concourse.tile as tile
from concourse import mybir
from concourse.bass_utils import run_bass_kernel_spmd
from concourse.dve_ops import DveOp
from concourse.dve_spec import (
    AluOp,
    Bin,
    C0,
    C1,
    Spec,
    Src0,
    Src1,
    Zero,
    _has_src1,
    lower,
)
from concourse.dve_uop import DveOpSpec

BF16 = ml_dtypes.bfloat16
F8 = ml_dtypes.float8_e4m3fn

B, C, H, W = 8, 16, 512, 512
HW = H * W           # 262144 pixels per batch/core
G = 8                # pixel groups per core
M = HW // G          # 32768 pixels per group (free-dim length)
P = G * C            # 128 partitions
NCHUNK = 16
N = M // NCHUNK      # 2048 pixels per outer tile (DMA/exp granularity)
NH = 1024            # pixels per PSUM-bound inner chunk
NHPER = N // NH      # inner chunks per outer tile
SMOOTH = 1.0
IGNORE_INDEX = 255

# minimax-linear fit of 1/t on [-4.5, -4] (the interval x*bitcast(~x) lands
# in for any positive fp32 x); relative error 1.81e-3
RECIP_A = -0.47108412121536725
RECIP_B = -0.05538388804827088

_CACHE: dict = {}


def _ref_recip_mul(in0, in1, c0, c1, c2):
    u = (~np.asarray(in0, np.float32).view(np.int32)).view(np.float32)
    t = (in0 * u).astype(np.float32)
    b = ((u * (c0 + c1 * t)) * in1).astype(np.float32)
    return b, b.reshape(b.shape[0], -1).sum(axis=-1, keepdims=True)


def _make_dve_op(name, spec):
    """Build a DveOp with computed uop shas and register it in dve_ops."""
    if name in dve_ops._SUB_OPCODE_FOR_NAME:
        return next(op for op in dve_ops.OPS if op.name == name)
    shas = {}
    for ver in ("v3", "v4"):
        tmp = DveOpSpec(
            name=name, opcode=0, uops=lower(spec, ver=ver), rd1_en=_has_src1(spec)
        )
        shas[ver] = tmp.sha(ver)
    op = DveOp(name, spec, subdim=False, uops_sha=shas)
    row = dve_ops._CUSTOM_DVE_ROW_BASE + len(dve_ops.OPS)
    assert row < 0x20
    dve_ops.OPS.append(op)
    dve_ops._SUB_OPCODE_FOR_NAME[name] = row
    dve_ops.CUSTOM_DVE_SPECS[name] = spec
    return op


_u = Bin(AluOp.BITWISE_NOT, Src0, Src0)
_t = Src0 * _u

RECIP_MUL_DICE = _make_dve_op(
    "RECIP_MUL_DICE",
    Spec(
        body=(_u * (C0 + C1 * _t)) * Src1,
        accum=add,
        accum_init=Zero,
        reference=_ref_recip_mul,
    ),
)


def _build():
    nc = bacc.Bacc("TRN2", target_bir_lowering=False, debug=False)
    bf = mybir.dt.bfloat16
    f32 = mybir.dt.float32

    f8 = mybir.dt.float8e4
    xp = nc.dram_tensor("xp", (P, M), f8, kind="ExternalInput").ap()
    lt = nc.dram_tensor("lt", (P, N), f8, kind="ExternalInput").ap()
    sel = nc.dram_tensor("sel", (P, P), bf, kind="ExternalInput").ap()
    # Group-selector ones [128, 8] embedded at columns 24..32 of a zero
    # [128, 56] band; the 32-wide window starting at col 24-8k is the
    # chunk-(4q+k) DT weight variant (selector at window cols 8k..8k+8, so
    # DT rows land at partition 32q+8k+g; PE out base must be 0/32/64/96
    # and the 4-chunk group accumulates into one 32-row PSUM block).
    w8 = nc.dram_tensor("w8", (P, 56), bf, kind="ExternalInput").ap()
    itr = nc.dram_tensor("itr", (P, N), bf, kind="ExternalOutput").ap()
    NACC = NHPER * NCHUNK
    accp = nc.dram_tensor("accp", (P, NACC), f32, kind="ExternalOutput").ap()

    with tile.TileContext(nc) as tc:
        with (
            tc.tile_pool(name="io", bufs=4) as io,
            tc.tile_pool(name="wt", bufs=1) as wt,
            tc.tile_pool(name="ps", bufs=2, space=bass.MemorySpace.PSUM) as ps,
            tc.tile_pool(name="pdt", bufs=1, space=bass.MemorySpace.PSUM) as pdt,
            tc.tile_pool(name="ac", bufs=1) as ac,
        ):
            selt = wt.tile([P, P], bf)
            w8t = wt.tile([P, 56], bf)
            ltt = wt.tile([P, N], f8)
            ET = ac.tile([P, N], bf)
            ITR = ac.tile([P, N], bf)
            scr = ac.tile([P, 1], f32)

            accP = ac.tile([P, NACC], f32)
            DTp = pdt.tile([P, N], f32)

            def dt_matmul(i, E, half=None):
                # compact per-pixel denominators for this chunk's pixels:
                # DTp[i*8+g, j] = sum_c E[g*16+c, j]. The 32-row block
                # 32*(i//4) accumulates chunks 4q..4q+3 via shifted zero-
                # padded weight columns (PE out base must be 0/32/64/96).
                q, k = divmod(i, 4)
                rng = (
                    range(0, N, 512)
                    if half is None
                    else range(half * NH, (half + 1) * NH, 512)
                )
                for s in rng:
                    nc.tensor.matmul(
                        DTp[32 * q : 32 * (q + 1), s : s + 512],
                        w8t[:, 24 - 8 * k : 56 - 8 * k],
                        E[:, s : s + 512],
                        start=(k == 0),
                        stop=(k == 3),
                        tile_position=(0, 32 * q),
                    )

            def recips(i, E, interleave_dt=True):
                # per EXP-half: 2 denominator matmuls + 2 DT matmuls back to
                # back on the PE (denser PE stream, hotter p-state), then the
                # fused recip+p_sum op on Vector
                for h in range(NHPER):
                    D = ps.tile([P, NH], f32, tag="D")
                    for s2 in range(0, NH, 512):
                        nc.tensor.matmul(
                            D[:, s2 : s2 + 512],
                            selt[:],
                            E[:, h * NH + s2 : h * NH + s2 + 512],
                            start=True,
                            stop=True,
                        )
                    if interleave_dt:
                        dt_matmul(i, E, half=h)
                    Pt = io.tile([P, NH], bf, tag="P")
                    nc.vector._custom_dve(
                        RECIP_MUL_DICE,
                        out=Pt[:],
                        in0=D[:],
                        in1=E[:, h * NH : (h + 1) * NH],
                        s0=RECIP_A,
                        s1=RECIP_B,
                        accum_out=accP[:, NHPER * i + h : NHPER * i + h + 1],
                    )

            for i in range(NCHUNK):
                sl = slice(i * N, (i + 1) * N)
                # halved DMA + EXP: shortens the produce->consume latency at
                # each pipeline stage (PE can start on h0 while h1 still exps)
                L = io.tile([P, N], f8, tag="L", bufs=6)
                if i == 0:
                    # chunk 0 leads the GpSimd ring (shortest preamble) so
                    # its halves hit the HW queues first; weights lead the
                    # Sync ring; Scalar ring stays clean for the table load
                    nc.gpsimd.dma_start(L[:, 0:NH], xp[:, 0:NH])
                    nc.gpsimd.dma_start(L[:, NH:N], xp[:, NH:N])
                    nc.sync.dma_start(selt[:], sel)
                    nc.sync.dma_start(w8t[:], w8)
                else:
                    nc.sync.dma_start(L[:, 0:NH], xp[:, i * N : i * N + NH])
                    nc.gpsimd.dma_start(
                        L[:, NH:N], xp[:, i * N + NH : (i + 1) * N]
                    )
                    if i == 1:
                        nc.gpsimd.dma_start(ltt[:], lt)

                E = io.tile([P, N], bf, tag="E", bufs=6)
                nc.scalar.activation(
                    E[:, 0:NH], L[:, 0:NH], mybir.ActivationFunctionType.Exp
                )
                nc.scalar.activation(
                    E[:, NH:N], L[:, NH:N], mybir.ActivationFunctionType.Exp
                )
                if i == 2:
                    # target-logit exp: ScalarE has slack mid-pipeline
                    nc.scalar.activation(
                        ET[:], ltt[:], mybir.ActivationFunctionType.Exp
                    )

                if i < NCHUNK - 1:
                    recips(i, E, interleave_dt=False)
                    dt_matmul(i, E)
                    if i == NCHUNK - 2:
                        # bulk of the p_sum partials ships while chunk 15
                        # runs; only 2 columns remain on the critical tail
                        nc.gpsimd.dma_start(
                            accp[:, 0 : NHPER * (NCHUNK - 1)],
                            accP[:, 0 : NHPER * (NCHUNK - 1)],
                        )
                else:
                    # last chunk: DTp completes first, then ITR + its DMA
                    # hide behind the final pair of recip ops
                    dt_matmul(i, E)
                    nc.vector._custom_dve(
                        RECIP_MUL_DICE,
                        out=ITR[:],
                        in0=DTp[:],
                        in1=ET[:],
                        s0=RECIP_A,
                        s1=RECIP_B,
                        accum_out=scr[:],
                    )
                    nc.gpsimd.dma_start(itr, ITR[:])
                    recips(i, E, interleave_dt=False)

            nc.gpsimd.dma_start(
                accp[:, NHPER * (NCHUNK - 1) :], accP[:, NHPER * (NCHUNK - 1) :]
            )

    nc.compile()
    return nc


def _get_nc():
    nc = _CACHE.get("nc")
    if nc is None:
        nc = _build()
        _CACHE["nc"] = nc
    return nc


def _host_inputs(logits, targets):
    sel_np = np.kron(
        np.eye(G, dtype=np.float32), np.ones((C, C), np.float32)
    ).astype(BF16)  # [128, 128] block-diag 16x16 ones
    w8_np = np.zeros((P, 56), np.float32)
    w8_np[:, 24:32] = np.kron(
        np.eye(G, dtype=np.float32), np.ones((C, 1), np.float32)
    )  # selector band: w8[g*16+c, 24+g] = 1
    w8_np = w8_np.astype(BF16)

    logits = np.asarray(logits)
    targets = np.asarray(targets)
    in_maps = []
    for b in range(B):
        lf = logits[b].reshape(C, HW)
        xp = lf.reshape(C, G, M).transpose(1, 0, 2).reshape(P, M).astype(F8)
        tflat = targets[b].reshape(HW).astype(np.int64)
        tsafe = np.where(tflat == IGNORE_INDEX, 0, tflat)
        # target-class logit per pixel, pixel-major rows q = i*8 + g
        ltf = lf[tsafe, np.arange(HW)]
        ltp = (
            ltf.reshape(G, NCHUNK, N).transpose(1, 0, 2).reshape(P, N).astype(F8)
        )
        in_maps.append({"xp": xp, "lt": ltp, "sel": sel_np, "w8": w8_np})
    return in_maps


def _combine(results, targets):
    targets = np.asarray(targets)
    Ps = np.zeros(C, np.float64)
    Ic = np.zeros(C, np.float64)
    Ts = np.zeros(C, np.float64)
    for b, r in enumerate(results):
        acc = r["accp"].astype(np.float64)  # [128, 32] p_sum partials
        Ps += acc.sum(axis=1).reshape(G, C).sum(axis=0)
        # itr rows q = i*8 + g, cols j -> pixel g*M + i*N + j
        itr = (
            r["itr"].astype(np.float64).reshape(NCHUNK, G, N).transpose(1, 0, 2)
        ).reshape(HW)
        tflat = targets[b].reshape(HW).astype(np.int64)
        valid = tflat != IGNORE_INDEX
        Ic += np.bincount(tflat[valid], weights=itr[valid], minlength=C)[:C]
        Ts += np.bincount(tflat[valid], minlength=C)[:C]
    dice = (2.0 * Ic + SMOOTH) / (Ps + Ts + SMOOTH)
    return np.float32(np.mean(1.0 - dice))


def kernel(logits, targets):
    nc = _get_nc()
    in_maps = _host_inputs(logits, targets)
    res = run_bass_kernel_spmd(nc, in_maps, list(range(B)))
    return _combine(res.results, targets)


if __name__ == "__main__":
    rng = np.random.default_rng(0)
    logits = rng.standard_normal((B, C, H, W), dtype=np.float32)
    targets = rng.integers(0, C, size=(B, H, W)).astype(np.int64)
    print("loss:", kernel(logits, targets))


# revision 43
# speedup vs baseline: 1.0148x; 1.0048x over previous
"""DiceLoss (multiclass, softmax over C=16) on 8 Trainium2 NeuronCores.

Data-parallel: batch b -> core b. Per core, logits [16, 512*512] are packed
as [128, 32768] fp8-e4m3 (softmax is insensitive to small logit noise; the
quantization error averages out over 16k+ pixels per class): partition
p = g*16 + c (g = pixel-group of 32768 pixels, c = class), free axis =
pixel-within-group. Per 2048-pixel chunk:

  E  = exp(L)                 ACT (the ONLY ScalarE op -> one table set)
  D  = SelRep.T @ E           PE -> PSUM f32 (SelRep = 16x16 block-diag ones
                              -> per-pixel softmax denominator, replicated to
                              all 16 class-partitions; constant weights)
  DT[8i:8i+8] = W8.T @ E      PE -> persistent PSUM f32 [128, 2048]: compact
                              per-pixel denominators, pixel-major (row i*8+g
                              holds group g's pixels of chunk i). Same E
                              stream, W8 = group-selector ones [128, 8].
  P  = E * approx(1/D)        custom DVE op RECIP_MUL_DICE, one instruction:
       p_sum += sum(P)        bitcast-NOT exponent-flip seed + minimax-linear
                              refine (~1.8e-3 rel err, cancels in the dice
                              ratio), fused in1 multiply + free-axis accum
                              (reads D straight from PSUM). Output tensor is
                              scratch -- only the accumulator is used.

Intersection needs only the TARGET-class probability per pixel. The host
sends lt [128, 2048] fp8 = the target-class logit per pixel (same fp8
values as in xp, pixel-major, row i*8+g). Tail:

  ET  = exp(lt)               ACT (bit-identical to the matching E entries)
  ITR = ET * approx(1/DT)     one more RECIP_MUL_DICE [128, 2048] -> per-pixel
                              target-class softmax prob, DMA'd back (512KB).

Host folds: p_sum from the accumulator columns, I_c = bincount(targets,
weights=ITR), t_sum = bincount(targets). No mask tensor, no mask multiply,
no intersection matmul stream. No on-device collective: per-core partials
combine on host.
"""

import sys

for _p in ("/opt/trn_rl_repo",):
    if _p not in sys.path:
        sys.path.insert(0, _p)

from operator import add

import numpy as np
import ml_dtypes

import concourse.bacc as bacc
import concourse.bass as bass
import concourse.dve_ops as dve_ops
import concourse.tile as tile
from concourse import mybir
from concourse.bass_utils import run_bass_kernel_spmd
from concourse.dve_ops import DveOp
from concourse.dve_spec import (
    AluOp,
    Bin,
    C0,
    C1,
    Spec,
    Src0,
    Src1,
    Zero,
    _has_src1,
    lower,
)
from concourse.dve_uop import DveOpSpec

BF16 = ml_dtypes.bfloat16
F8 = ml_dtypes.float8_e4m3fn

B, C, H, W = 8, 16, 512, 512
HW = H * W           # 262144 pixels per batch/core
G = 8                # pixel groups per core
M = HW // G          # 32768 pixels per group (free-dim length)
P = G * C            # 128 partitions
NCHUNK = 16
N = M // NCHUNK      # 2048 pixels per outer tile (DMA/exp granularity)
NH = 1024            # pixels per PSUM-bound inner chunk
NHPER = N // NH      # inner chunks per outer tile
SMOOTH = 1.0
IGNORE_INDEX = 255

# minimax-linear fit of 1/t on [-4.5, -4] (the interval x*bitcast(~x) lands
# in for any positive fp32 x); relative error 1.81e-3
RECIP_A = -0.47108412121536725
RECIP_B = -0.05538388804827088

_CACHE: dict = {}


def _ref_recip_mul(in0, in1, c0, c1, c2):
    u = (~np.asarray(in0, np.float32).view(np.int32)).view(np.float32)
    t = (in0 * u).astype(np.float32)
    b = ((u * (c0 + c1 * t)) * in1).astype(np.float32)
    return b, b.reshape(b.shape[0], -1).sum(axis=-1, keepdims=True)


def _make_dve_op(name, spec):
    """Build a DveOp with computed uop shas and register it in dve_ops."""
    if name in dve_ops._SUB_OPCODE_FOR_NAME:
        return next(op for op in dve_ops.OPS if op.name == name)
    shas = {}
    for ver in ("v3", "v4"):
        tmp = DveOpSpec(
            name=name, opcode=0, uops=lower(spec, ver=ver), rd1_en=_has_src1(spec)
        )
        shas[ver] = tmp.sha(ver)
    op = DveOp(name, spec, subdim=False, uops_sha=shas)
    row = dve_ops._CUSTOM_DVE_ROW_BASE + len(dve_ops.OPS)
    assert row < 0x20
    dve_ops.OPS.append(op)
    dve_ops._SUB_OPCODE_FOR_NAME[name] = row
    dve_ops.CUSTOM_DVE_SPECS[name] = spec
    return op


_u = Bin(AluOp.BITWISE_NOT, Src0, Src0)
_t = Src0 * _u

RECIP_MUL_DICE = _make_dve_op(
    "RECIP_MUL_DICE",
    Spec(
        body=(_u * (C0 + C1 * _t)) * Src1,
        accum=add,
        accum_init=Zero,
        reference=_ref_recip_mul,
    ),
)


def _build():
    nc = bacc.Bacc("TRN2", target_bir_lowering=False, debug=False)
    bf = mybir.dt.bfloat16
    f32 = mybir.dt.float32

    f8 = mybir.dt.float8e4
    xp = nc.dram_tensor("xp", (P, M), f8, kind="ExternalInput").ap()
    lt = nc.dram_tensor("lt", (P, N), f8, kind="ExternalInput").ap()
    sel = nc.dram_tensor("sel", (P, P), bf, kind="ExternalInput").ap()
    # Group-selector ones [128, 8] embedded at columns 24..32 of a zero
    # [128, 56] band; the 32-wide window starting at col 24-8k is the
    # chunk-(4q+k) DT weight variant (selector at window cols 8k..8k+8, so
    # DT rows land at partition 32q+8k+g; PE out base must be 0/32/64/96
    # and the 4-chunk group accumulates into one 32-row PSUM block).
    w8 = nc.dram_tensor("w8", (P, 56), bf, kind="ExternalInput").ap()
    itr = nc.dram_tensor("itr", (P, N), bf, kind="ExternalOutput").ap()
    NACC = NHPER * NCHUNK
    accp = nc.dram_tensor("accp", (P, NACC), f32, kind="ExternalOutput").ap()

    with tile.TileContext(nc) as tc:
        with (
            tc.tile_pool(name="io", bufs=4) as io,
            tc.tile_pool(name="wt", bufs=1) as wt,
            tc.tile_pool(name="ps", bufs=2, space=bass.MemorySpace.PSUM) as ps,
            tc.tile_pool(name="pdt", bufs=1, space=bass.MemorySpace.PSUM) as pdt,
            tc.tile_pool(name="ac", bufs=1) as ac,
        ):
            selt = wt.tile([P, P], bf)
            w8t = wt.tile([P, 56], bf)
            ltt = wt.tile([P, N], f8)
            ET = ac.tile([P, N], bf)
            ITR = ac.tile([P, N], bf)
            scr = ac.tile([P, 1], f32)

            accP = ac.tile([P, NACC], f32)
            DTp = pdt.tile([P, N], f32)

            def dt_matmul(i, E, half=None):
                # compact per-pixel denominators for this chunk's pixels:
                # DTp[i*8+g, j] = sum_c E[g*16+c, j]. The 32-row block
                # 32*(i//4) accumulates chunks 4q..4q+3 via shifted zero-
                # padded weight columns (PE out base must be 0/32/64/96).
                q, k = divmod(i, 4)
                rng = (
                    range(0, N, 512)
                    if half is None
                    else range(half * NH, (half + 1) * NH, 512)
                )
                for s in rng:
                    nc.tensor.matmul(
                        DTp[32 * q : 32 * (q + 1), s : s + 512],
                        w8t[:, 24 - 8 * k : 56 - 8 * k],
                        E[:, s : s + 512],
                        start=(k == 0),
                        stop=(k == 3),
                        tile_position=(0, 32 * q),
                    )

            def recips(i, E, interleave_dt=True):
                # per EXP-half: 2 denominator matmuls + 2 DT matmuls back to
                # back on the PE (denser PE stream, hotter p-state), then the
                # fused recip+p_sum op on Vector
                for h in range(NHPER):
                    D = ps.tile([P, NH], f32, tag="D")
                    for s2 in range(0, NH, 512):
                        nc.tensor.matmul(
                            D[:, s2 : s2 + 512],
                            selt[:],
                            E[:, h * NH + s2 : h * NH + s2 + 512],
                            start=True,
                            stop=True,
                        )
                    if interleave_dt:
                        dt_matmul(i, E, half=h)
                    Pt = io.tile([P, NH], bf, tag="P")
                    nc.vector._custom_dve(
                        RECIP_MUL_DICE,
                        out=Pt[:],
                        in0=D[:],
                        in1=E[:, h * NH : (h + 1) * NH],
                        s0=RECIP_A,
                        s1=RECIP_B,
                        accum_out=accP[:, NHPER * i + h : NHPER * i + h + 1],
                    )

            for i in range(NCHUNK):
                sl = slice(i * N, (i + 1) * N)
                # halved DMA + EXP: shortens the produce->consume latency at
                # each pipeline stage (PE can start on h0 while h1 still exps)
                L = io.tile([P, N], f8, tag="L", bufs=6)
                if i == 0:
                    # chunk 0 leads the GpSimd ring (shortest preamble) so
                    # its halves hit the HW queues first; weights lead the
                    # Sync ring; Scalar ring stays clean for the table load
                    nc.gpsimd.dma_start(L[:, 0:NH], xp[:, 0:NH])
                    nc.gpsimd.dma_start(L[:, NH:N], xp[:, NH:N])
                    nc.sync.dma_start(selt[:], sel)
                    nc.sync.dma_start(w8t[:], w8)
                else:
                    nc.sync.dma_start(L[:, 0:NH], xp[:, i * N : i * N + NH])
                    nc.gpsimd.dma_start(
                        L[:, NH:N], xp[:, i * N + NH : (i + 1) * N]
                    )
                    if i == 1:
                        nc.gpsimd.dma_start(ltt[:], lt)

                if i == 0:
                    # PE p-state warmup: dummy matmuls into the DTp block
                    # while waiting for the first exp (the real chunk-0 DT
                    # group starts with start=True, overwriting the garbage).
                    # ~3us of continuous PE busy ramps the clock to full
                    # before the first real denominator matmul.
                    for _ in range(12):
                        nc.tensor.matmul(
                            DTp[0:32, 0:128],
                            selt[:, 0:32],
                            selt[:, 0:128],
                            start=True,
                            stop=True,
                            tile_position=(0, 0),
                        )

                E = io.tile([P, N], bf, tag="E", bufs=6)
                nc.scalar.activation(
                    E[:, 0:NH], L[:, 0:NH], mybir.ActivationFunctionType.Exp
                )
                nc.scalar.activation(
                    E[:, NH:N], L[:, NH:N], mybir.ActivationFunctionType.Exp
                )
                if i == 2:
                    # target-logit exp: ScalarE has slack mid-pipeline
                    nc.scalar.activation(
                        ET[:], ltt[:], mybir.ActivationFunctionType.Exp
                    )

                if i < NCHUNK - 1:
                    recips(i, E, interleave_dt=False)
                    dt_matmul(i, E)
                    if i == NCHUNK - 2:
                        # bulk of the p_sum partials ships while chunk 15
                        # runs; only 2 columns remain on the critical tail
                        nc.gpsimd.dma_start(
                            accp[:, 0 : NHPER * (NCHUNK - 1)],
                            accP[:, 0 : NHPER * (NCHUNK - 1)],
                        )
                else:
                    # last chunk: DTp completes first, then ITR + its DMA
                    # hide behind the final pair of recip ops
                    dt_matmul(i, E)
                    nc.vector._custom_dve(
                        RECIP_MUL_DICE,
                        out=ITR[:],
                        in0=DTp[:],
                        in1=ET[:],
                        s0=RECIP_A,
                        s1=RECIP_B,
                        accum_out=scr[:],
                    )
                    nc.gpsimd.dma_start(itr, ITR[:])
                    recips(i, E, interleave_dt=False)

            nc.gpsimd.dma_start(
                accp[:, NHPER * (NCHUNK - 1) :], accP[:, NHPER * (NCHUNK - 1) :]
            )

    nc.compile()
    return nc


def _get_nc():
    nc = _CACHE.get("nc")
    if nc is None:
        nc = _build()
        _CACHE["nc"] = nc
    return nc


def _host_inputs(logits, targets):
    sel_np = np.kron(
        np.eye(G, dtype=np.float32), np.ones((C, C), np.float32)
    ).astype(BF16)  # [128, 128] block-diag 16x16 ones
    w8_np = np.zeros((P, 56), np.float32)
    w8_np[:, 24:32] = np.kron(
        np.eye(G, dtype=np.float32), np.ones((C, 1), np.float32)
    )  # selector band: w8[g*16+c, 24+g] = 1
    w8_np = w8_np.astype(BF16)

    logits = np.asarray(logits)
    targets = np.asarray(targets)
    in_maps = []
    for b in range(B):
        lf = logits[b].reshape(C, HW)
        xp = lf.reshape(C, G, M).transpose(1, 0, 2).reshape(P, M).astype(F8)
        tflat = targets[b].reshape(HW).astype(np.int64)
        tsafe = np.where(tflat == IGNORE_INDEX, 0, tflat)
        # target-class logit per pixel, pixel-major rows q = i*8 + g
        ltf = lf[tsafe, np.arange(HW)]
        ltp = (
            ltf.reshape(G, NCHUNK, N).transpose(1, 0, 2).reshape(P, N).astype(F8)
        )
        in_maps.append({"xp": xp, "lt": ltp, "sel": sel_np, "w8": w8_np})
    return in_maps


def _combine(results, targets):
    targets = np.asarray(targets)
    Ps = np.zeros(C, np.float64)
    Ic = np.zeros(C, np.float64)
    Ts = np.zeros(C, np.float64)
    for b, r in enumerate(results):
        acc = r["accp"].astype(np.float64)  # [128, 32] p_sum partials
        Ps += acc.sum(axis=1).reshape(G, C).sum(axis=0)
        # itr rows q = i*8 + g, cols j -> pixel g*M + i*N + j
        itr = (
            r["itr"].astype(np.float64).reshape(NCHUNK, G, N).transpose(1, 0, 2)
        ).reshape(HW)
        tflat = targets[b].reshape(HW).astype(np.int64)
        valid = tflat != IGNORE_INDEX
        Ic += np.bincount(tflat[valid], weights=itr[valid], minlength=C)[:C]
        Ts += np.bincount(tflat[valid], minlength=C)[:C]
    dice = (2.0 * Ic + SMOOTH) / (Ps + Ts + SMOOTH)
    return np.float32(np.mean(1.0 - dice))


def kernel(logits, targets):
    nc = _get_nc()
    in_maps = _host_inputs(logits, targets)
    res = run_bass_kernel_spmd(nc, in_maps, list(range(B)))
    return _combine(res.results, targets)


if __name__ == "__main__":
    rng = np.random.default_rng(0)
    logits = rng.standard_normal((B, C, H, W), dtype=np.float32)
    targets = rng.integers(0, C, size=(B, H, W)).astype(np.int64)
    print("loss:", kernel(logits, targets))
